# revision 41
# baseline (speedup 1.0000x reference)
"""Causal self-attention Trainium2 kernel (8 NeuronCores, SPMD).

Sharding: 8 cores = 2 batches x 4 head-groups (4 heads of 64 dims each).
Each core computes full-sequence attention for its 4 heads plus the
partial output projection for its 256 y-columns; the host sums the 4
bf16 partials per batch (in f32) and adds the output bias.

Layout strategy (no on-device transposes anywhere):
  - host supplies x[b].T as xT [C, T] (bf16)
  - qT, kT produced in [m, t] layout (W stationary reused across 4
    t-chunks per contraction tile; biases added on ScalarE)
  - v produced in natural [t, m] layout, augmented with a ones column
    per head (M=65) so the attention-value matmul also emits the
    softmax denominator row for free
  - attT[j, i] = sum_d kT[d,j] qT[d,i] (kT stationary K=64; two heads
    run concurrently via inferred 64-row PE array tiling)
  - exp on ScalarE (fused 1/sqrt(64) scale); diagonal 128x128 blocks
    masked multiplicatively on GpSimd
  - probability strips stored triangularly packed: strip jt holds
    columns [128*jt, T), written once at its diagonal i-block
  - i-block-OUTER pipeline: per ib, for each head pair: new score
    strips + exp, then attT@V accumulation over all ready strips
    (old strips first so the PE never waits on fresh exp), then
    off-PE normalization (approx reciprocal + partition_broadcast);
    out-projection of ib-1 is emitted between ib's score and value
    phases as dense PE filler while ScalarE runs exp
  - out[t, n] partial in bf16, DMA'd out per 128x512 tile
"""

import sys

for _p in ("/opt/trn_rl_repo",):
    if _p not in sys.path:
        sys.path.insert(0, _p)

from contextlib import ExitStack

import ml_dtypes
import numpy as np

import concourse.bass as bass
import concourse.tile as tile
from concourse import bacc, mybir
from concourse.bass_utils import run_bass_kernel_spmd

BF16 = mybir.dt.bfloat16
F32 = mybir.dt.float32
NP_BF16 = ml_dtypes.bfloat16

B, T, C = 2, 2048, 1024
H, D = 16, 64
N_CORES = 8
GROUPS = 4          # head groups (cores per batch)
MH = C // GROUPS    # 256 columns per core (4 heads)
LH = MH // D        # 4 local heads
CT = C // 128       # 8 contraction tiles
TT = T // 128       # 16 sequence tiles of 128
IB = T // 512       # 4 i-blocks of 512
SCALE = 1.0 / np.sqrt(D)

# triangular strip offsets: strip jt holds columns [128*jt, T)
OFFS = [0] * TT
for _jt in range(1, TT):
    OFFS[_jt] = OFFS[_jt - 1] + (T - 128 * (_jt - 1))
PTW = OFFS[TT - 1] + (T - 128 * (TT - 1))  # 17408


def _causal_mask() -> np.ndarray:
    """mask[j, i] = 1.0 if j <= i else 0 (bf16), [128, 128]."""
    j = np.arange(128)[:, None]
    i = np.arange(128)[None, :]
    return (j <= i).astype(NP_BF16)


def emit_kernel(nc, xT_d, wq_d, wk_d, wv_d, wp_d, bq_d, bk_d, bv_d, out_d, mask_d):
    with tile.TileContext(nc) as tc, ExitStack() as ctx:
        # ---- long-lived tiles -------------------------------------------
        keep = ctx.enter_context(tc.tile_pool(name="keep", bufs=1))
        qT_s = keep.tile([128, 2, T], BF16, tag="qT")
        kT_s = keep.tile([128, 2, T], BF16, tag="kT")
        v_s = keep.tile([128, TT, LH, D + 1], BF16, tag="v")
        yTn_s = keep.tile([128, 2, T], BF16, tag="yTn")
        wp_s = keep.tile([128, 2, C], BF16, tag="wp")
        mask_st = keep.tile([128, 128], BF16, tag="mask_st")
        mask_s = keep.tile([128, 128], BF16, tag="mask")
        bq_st = keep.tile([128, 2], F32, tag="bq_st")
        bq_s = keep.tile([128, 2], F32, tag="bq")
        bk_st = keep.tile([128, 2], F32, tag="bk_st")
        bk_s = keep.tile([128, 2], F32, tag="bk")
        bv_row = keep.tile([1, MH], F32, tag="bv_row")
        bv_row_bf = keep.tile([1, MH], BF16, tag="bv_row_bf")
        bv_bc = keep.tile([128, MH], F32, tag="bv_bc")
        ones_bf128 = keep.tile([1, 128], BF16, tag="ones_bf128")

        nc.vector.memset(ones_bf128[:], 1.0)
        nc.vector.memset(v_s[:, :, :, D : D + 1], 1.0)

        # ---- phase 1: projections --------------------------------------
        # pta/aps opened on the outer stack (tiles allocated later) so the
        # inner proj pools can close around them in LIFO order
        pta = ctx.enter_context(tc.tile_pool(name="pta", bufs=1))
        aps = ctx.enter_context(tc.tile_pool(name="att_ps", bufs=2, space="PSUM"))
        pin_es = ExitStack()
        pin = pin_es.enter_context(tc.tile_pool(name="proj_in", bufs=1))
        pps_es = ExitStack()
        pps = pps_es.enter_context(
            tc.tile_pool(name="proj_ps", bufs=4, space="PSUM")
        )
        if True:
            xT_s = pin.tile([128, CT, T], BF16, tag="xT")
            wq_s = pin.tile([128, CT, MH], BF16, tag="wq")
            wk_s = pin.tile([128, CT, MH], BF16, tag="wk")
            wv_s = pin.tile([128, CT, MH], BF16, tag="wv")
            # SP issues DMAs in program order: first-needed data first
            xT_r = xT_d.ap().rearrange("(o p) t -> p o t", p=128)
            wq_r = wq_d.ap().rearrange("(o p) m -> p o m", p=128)
            wk_r = wk_d.ap().rearrange("(o p) m -> p o m", p=128)
            wv_r = wv_d.ap().rearrange("(o p) m -> p o m", p=128)

            def xt_chunk(tb):
                nc.sync.dma_start(
                    xT_s[:, :, tb * 512 : (tb + 1) * 512],
                    xT_r[:, :, tb * 512 : (tb + 1) * 512],
                )

            nc.sync.dma_start(wq_s[:], wq_r[:])
            xt_chunk(0)
            nc.sync.dma_start(wk_s[:], wk_r[:])
            xt_chunk(1)
            nc.sync.dma_start(wv_s[:], wv_r[:])
            xt_chunk(2)
            xt_chunk(3)
            wp_r = wp_d.ap().rearrange("(o p) n -> p o n", p=128)
            nc.sync.dma_start(wp_s[:], wp_r[:])
            # consts staged through a DVE copy: consumers then depend on DVE
            # program order instead of a DMA semaphore (walrus 1-wait limit)
            nc.gpsimd.dma_start(mask_st[:], mask_d.ap())
            nc.gpsimd.dma_start(bq_st[:], bq_d.ap().rearrange("(o p) -> p o", p=128))
            nc.gpsimd.dma_start(bk_st[:], bk_d.ap().rearrange("(o p) -> p o", p=128))
            nc.gpsimd.dma_start(bv_row[:], bv_d.ap()[None, :])
            nc.vector.tensor_copy(mask_s[:], mask_st[:])
            nc.vector.tensor_copy(bq_s[:], bq_st[:])
            nc.vector.tensor_copy(bk_s[:], bk_st[:])
            nc.vector.tensor_copy(bv_row_bf[:], bv_row[:])

            # qT / kT: [m, t] = W.T @ x.T  (W stationary, reused over 4
            # t-chunks; 4 PSUM banks live per (w, mt) round)
            for w_s, b_s, dst in ((wq_s, bq_s, qT_s), (wk_s, bk_s, kT_s)):
                for mt in range(2):
                    pss = [
                        pps.tile([128, 512], F32, tag="proj_ps", name=f"pss{_tb}")
                        for _tb in range(IB)
                    ]
                    for ct in range(CT):
                        for tb in range(IB):
                            nc.tensor.matmul(
                                pss[tb][:],
                                w_s[:, ct, mt * 128 : (mt + 1) * 128],
                                xT_s[:, ct, tb * 512 : (tb + 1) * 512],
                                start=(ct == 0),
                                stop=(ct == CT - 1),
                            )
                    for tb in range(IB):
                        nc.scalar.add(
                            dst[:, mt, tb * 512 : (tb + 1) * 512],
                            pss[tb][:],
                            b_s[:, mt : mt + 1],
                        )

            pps_es.close()
            vps_es = ExitStack()
            vps = vps_es.enter_context(
                tc.tile_pool(name="v_psp", bufs=4, space="PSUM")
            )
            bv_ps = vps.tile([128, MH], F32, tag="v_ps", name="bv_ps")
            nc.tensor.matmul(
                bv_ps[:], ones_bf128[:], bv_row_bf[:], start=True, stop=True
            )
            nc.vector.tensor_copy(bv_bc[:], bv_ps[:])

            # v natural [t, m]  (xT stationary)
            def v_tt(tt):
                ps = vps.tile([128, MH], F32, tag="v_ps", name="v_ps")
                for ct in range(CT):
                    nc.tensor.matmul(
                        ps[:],
                        xT_s[:, ct, tt * 128 : (tt + 1) * 128],
                        wv_s[:, ct, :],
                        start=(ct == 0),
                        stop=(ct == CT - 1),
                    )
                nc.vector.tensor_tensor(
                    v_s[:, tt, :, 0:D],
                    ps[:].rearrange("p (h d) -> p h d", h=LH),
                    bv_bc[:].rearrange("p (h d) -> p h d", h=LH),
                    mybir.AluOpType.add,
                )

            for tt in range(4):
                v_tt(tt)

        # ---- phase 2+3: attention, ib-outer pipeline ---------------------
        # PTa (strips 0-7) coexists with xT so ib0 scores can interleave
        # with the tail of the v projection as dense PE filler
        PTa = [
            [
                pta.tile(
                    [128, OFFS[8]], BF16, tag=f"PTa{p}{lh}", name=f"PTa{p}{lh}"
                )
                for lh in range(2)
            ]
            for p in range(2)
        ]
        PTb = None

        def pt_slice(p, lh, jt):
            if jt < 8:
                return PTa[p][lh], OFFS[jt]
            return PTb[p][lh], OFFS[jt] - OFFS[8]

        def scores_strip(p, jt):
            # full future strip [ia, T) of pair p, chunked at 1024
            ia = 128 * jt
            ptile, base = pt_slice(p, 0, jt)
            off = 0
            w_all = T - ia
            while off < w_all:
                cw = min(1024, w_all - off)
                for lh in range(2):
                    ptile, base = pt_slice(p, lh, jt)
                    att_ps = aps.tile(
                        [128, 1024], F32, tag="att_ps", name="att_ps"
                    )
                    prow = slice(64 * lh, 64 * lh + 64)
                    for s5 in range(0, cw, 512):
                        nn = min(512, cw - s5)
                        nc.tensor.matmul(
                            att_ps[:, s5 : s5 + nn],
                            kT_s[prow, p, jt * 128 : (jt + 1) * 128],
                            qT_s[prow, p, ia + off + s5 : ia + off + s5 + nn],
                            start=True,
                            stop=True,
                        )
                    nc.scalar.activation(
                        ptile[:, base + off : base + off + cw],
                        att_ps[:, :cw],
                        mybir.ActivationFunctionType.Exp,
                        scale=float(SCALE),
                    )
                    if off == 0:
                        # diagonal 128x128 tile: zero j > i
                        nc.vector.tensor_tensor(
                            ptile[:, base : base + 128],
                            ptile[:, base : base + 128],
                            mask_s[:],
                            mybir.AluOpType.mult,
                        )
                off += cw

        # interleave ib0 scores (exp-bound) with remaining v tiles (dense)
        vq = list(range(4, TT))
        for s, (p0, jt0) in enumerate(
            [(p, jt) for p in range(2) for jt in range(4)]
        ):
            scores_strip(p0, jt0)
            if vq:
                v_tt(vq.pop(0))
        while vq:
            v_tt(vq.pop(0))
        vps_es.close()
        pin_es.close()

        with (
            tc.tile_pool(name="ptb", bufs=1) as ptbp,
            tc.tile_pool(name="yt_ps", bufs=2, space="PSUM") as yps,
            tc.tile_pool(name="mix_ps", bufs=2, space="PSUM") as mixp,
            tc.tile_pool(name="norm", bufs=2) as npool,
            tc.tile_pool(name="out_sb", bufs=4) as osb,
        ):
            out_r = out_d.ap().rearrange("(tt p) n -> tt p n", p=128)
            PTb = [
                [
                    ptbp.tile(
                        [128, PTW - OFFS[8]], BF16,
                        tag=f"PTb{p}{lh}", name=f"PTb{p}{lh}",
                    )
                    for lh in range(2)
                ]
                for p in range(2)
            ]

            def outproj(ib, evac="vector"):
                for tt in range(4 * ib, 4 * ib + 4):
                    for nb in range(2):
                        o_ps = mixp.tile([128, 512], F32, tag="mix", name="o_ps")
                        for pp in range(2):
                            nc.tensor.matmul(
                                o_ps[:],
                                yTn_s[:, pp, tt * 128 : (tt + 1) * 128],
                                wp_s[:, pp, nb * 512 : (nb + 1) * 512],
                                start=(pp == 0),
                                stop=(pp == 1),
                            )
                        ot = osb.tile([128, 512], BF16, tag="out_t")
                        if evac == "vector":
                            nc.vector.tensor_copy(ot[:], o_ps[:])
                        else:
                            nc.scalar.copy(ot[:], o_ps[:])
                        nc.sync.dma_start(
                            out_r[tt, :, nb * 512 : (nb + 1) * 512], ot[:]
                        )

            for ib in range(IB):
                # scores for both pairs first: exp(p0) hides under
                # outproj(ib-1) + scores(p1); exp(p1) under attV(p0)
                # (ib0 strips were computed during the v-projection tail)
                if ib > 0:
                    for jt in range(4 * ib, 4 * ib + 4):
                        scores_strip(0, jt)
                    outproj(ib - 1)
                    for jt in range(4 * ib, 4 * ib + 4):
                        scores_strip(1, jt)
                for p in range(2):
                    # (b) attT@V accumulation, oldest strips first
                    yT_ps = [
                        yps.tile([D + 1, 512], F32, tag="yT_ps", name=f"yT_ps{lh}")
                        for lh in range(2)
                    ]
                    for jt in range(4 * ib + 4):
                        for lh in range(2):
                            ia = 128 * jt
                            c0 = max(512 * ib, ia)
                            ptile, base = pt_slice(p, lh, jt)
                            nc.tensor.matmul(
                                yT_ps[lh][:, c0 - 512 * ib : 512],
                                v_s[:, jt, 2 * p + lh, :],
                                ptile[
                                    :,
                                    base + c0 - ia : base + 512 * ib + 512 - ia,
                                ],
                                start=(jt == 0),
                                stop=(jt == 4 * ib + 3),
                            )
                    # (c) normalization, fully off the PE
                    srows = npool.tile([1, 2, 512], F32, tag="srows", name="srows")
                    yTu = npool.tile([64, 2, 512], BF16, tag="yTu", name="yTu")
                    # small denominator copies first: the recip chain is the
                    # longest downstream path, start it before the big stashes
                    for lh in range(2):
                        nc.vector.tensor_copy(
                            srows[:, lh, :], yT_ps[lh][D : D + 1, :]
                        )
                    nc.vector.reciprocal_approx_fast(srows[:], srows[:])
                    rs2 = npool.tile([1, 2, 512], BF16, tag="rs2", name="rs2")
                    with nc.allow_low_precision(
                        reason="1/s broadcast via bf16 matmul; bf16 noise ~0.4% ok"
                    ):
                        nc.vector.tensor_copy(rs2[:], srows[:])
                    for lh in range(2):
                        nc.vector.tensor_copy(yTu[:, lh, :], yT_ps[lh][0:D, :])
                    for lh in range(2):
                        # broadcast 1/s row to 64 partitions: ones outer product
                        S_t = mixp.tile([128, 512], F32, tag="mix", name="S_ps")
                        nc.tensor.matmul(
                            S_t[0:64, :],
                            ones_bf128[:, 0:64],
                            rs2[:, lh, :],
                            start=True,
                            stop=True,
                        )
                        nc.vector.tensor_tensor(
                            yTn_s[
                                64 * lh : 64 * lh + 64, p,
                                512 * ib : 512 * ib + 512,
                            ],
                            yTu[:, lh, :],
                            S_t[0:64, :],
                            mybir.AluOpType.mult,
                        )
            outproj(IB - 1, evac="scalar")


_NC_CACHE = None


def get_nc() -> bass.Bass:
    global _NC_CACHE
    if _NC_CACHE is None:
        nc = bacc.Bacc()
        xT_d = nc.declare_dram_parameter("xT", [C, T], BF16, isOutput=False)
        wq_d = nc.declare_dram_parameter("wq", [C, MH], BF16, isOutput=False)
        wk_d = nc.declare_dram_parameter("wk", [C, MH], BF16, isOutput=False)
        wv_d = nc.declare_dram_parameter("wv", [C, MH], BF16, isOutput=False)
        wp_d = nc.declare_dram_parameter("wp", [MH, C], BF16, isOutput=False)
        bq_d = nc.declare_dram_parameter("bq", [MH], F32, isOutput=False)
        bk_d = nc.declare_dram_parameter("bk", [MH], F32, isOutput=False)
        bv_d = nc.declare_dram_parameter("bv", [MH], F32, isOutput=False)
        out_d = nc.declare_dram_parameter("out", [T, C], BF16, isOutput=True)
        mask_d = nc.inline_tensor(_causal_mask(), name="causal_mask")
        emit_kernel(
            nc, xT_d, wq_d, wk_d, wv_d, wp_d, bq_d, bk_d, bv_d, out_d, mask_d
        )
        nc.finalize()
        _NC_CACHE = nc
    return _NC_CACHE


def make_in_maps(x, Wq, bq, Wk, bk, Wv, bv, Wp, bp):
    in_maps = []
    for core in range(N_CORES):
        b, g = divmod(core, GROUPS)
        sl = slice(g * MH, (g + 1) * MH)
        in_maps.append(
            {
                "xT": np.ascontiguousarray(x[b].T).astype(NP_BF16),
                "wq": np.ascontiguousarray(Wq[:, sl]).astype(NP_BF16),
                "wk": np.ascontiguousarray(Wk[:, sl]).astype(NP_BF16),
                "wv": np.ascontiguousarray(Wv[:, sl]).astype(NP_BF16),
                "wp": np.ascontiguousarray(Wp[sl, :]).astype(NP_BF16),
                "bq": np.ascontiguousarray(bq[sl]).astype(np.float32),
                "bk": np.ascontiguousarray(bk[sl]).astype(np.float32),
                "bv": np.ascontiguousarray(bv[sl]).astype(np.float32),
            }
        )
    return in_maps


def kernel(x, Wq, bq, Wk, bk, Wv, bv, Wp, bp, _results_hook=None, _trace=False):
    x = np.asarray(x, dtype=np.float32)
    nc = get_nc()
    in_maps = make_in_maps(x, Wq, bq, Wk, bk, Wv, bv, Wp, bp)
    res = run_bass_kernel_spmd(
        nc, in_maps, core_ids=list(range(N_CORES)), trace=_trace
    )
    if _results_hook is not None:
        _results_hook(res)
    out = np.zeros((B, T, C), dtype=np.float32)
    for core in range(N_CORES):
        b = core // GROUPS
        out[b] += res.results[core]["out"].astype(np.float32)
    out += np.asarray(bp, dtype=np.float32)[None, None, :]
    return out
